# revision 21
# baseline (speedup 1.0000x reference)
"""MixtureOfExpertsTreeEnsemble Trainium2 kernel (8-core SPMD, batch data-parallel).

Math (per batch row b, tree t):
  g[b,n,t] = sigmoid(x[b] @ W[n,:,t] + bias[n,t])          63 internal nodes
  p[b,l,t] = prod of g / (1-g) along root->leaf path        64 leaves
  w[l,d,t] = leaf_weight[l,d,t] * softmax_t(gates[l,d,t])
  out[b,d] = sum_{l,t} p[b,l,t] * w[l,d,t]

Sharding: batch 4096 -> 8 cores x 512 rows; node weights / leaf tables are
replicated (small).  No collectives; host concatenates the per-core outputs.

Device-side structure (per core):
  * all streamed operands are bf16 (the gates are bf16 downstream anyway, so
    f32 logits precision would be wasted); PSUM accumulation stays f32
  * phase A (PE):   logits as [b_tile(128), (node,tree)] bf16 matmuls.
    Loop is (btile-pair, chunk): each (node,tree) chunk is consumed as soon
    as its DMA lands, and one [128,1008] 2-bank PSUM tile serves two batch
    tiles -> half the sigmoid instructions on ACT
  * phase B (DVE):  leaf path probabilities by level doubling in a *block*
    layout (children stored [left | right]) so every op is contiguous and
    bf16 (DVE 2x mode); host pre-permutes node order (bit-reversal within
    each level) and the leaf tables to match
  * phase C (PE):   p transposed to [(leaf,tree), b] bf16 chunks, 4 chunks
    per PSUM bank, one DVE copy per bank
  * phase 0:        w = leaf_weight * softmax(gates) with (l,d) on partitions
    and t free: exp on ACT, reduce on DVE, the 64 normalize ops on the idle
    Pool engine; PE transposes are emitted after the batch loop so they
    never block phase A on the leaf-table DMAs
  * phase D (PE):   out_T[d,b] = sum_chunks w_T.T @ p_T, host transposes back
  * DMA: weight matrix on the SP HW-DGE ring; x / leaf tables / output on the
    ACT ring so the two streams overlap
"""

import sys

sys.path.insert(0, "/opt/trn_rl_repo")

import ml_dtypes
import numpy as np

BF16 = np.dtype(ml_dtypes.bfloat16)

MAX_DEPTH = 6
NUM_TREES = 64
LEAF_DIMS = 128
D_IN = 512
BATCH = 4096
N_INTERNAL = 63
N_LEAVES = 64
N_CORES = 8
BS = BATCH // N_CORES          # 512 batch rows per core
KT = D_IN // 128               # 4 contraction tiles
NT = N_INTERNAL * NUM_TREES    # 4032 (node,tree) pairs
NCHUNK = 8
CHUNK = NT // NCHUNK           # 504
NBT = BS // 128                # 4 batch tiles per core
NPT = N_LEAVES * NUM_TREES // 128  # 32 transpose chunks of (leaf,tree)


def _bitrev(x: int, bits: int) -> int:
    r = 0
    for _ in range(bits):
        r = (r << 1) | (x & 1)
        x >>= 1
    return r


# block-recursion orderings (see module docstring)
_NODES_PERM = np.array(
    [(2**lvl - 1) + _bitrev(j, lvl) for lvl in range(MAX_DEPTH) for j in range(2**lvl)]
)
_LEAF_PERM = np.array([_bitrev(j, MAX_DEPTH) for j in range(N_LEAVES)])

_BUILT = {}


def _build(use_bias: bool):
    """Build + finalize the per-core Bass program."""
    import concourse.bacc as bacc
    import concourse.tile as tile
    from concourse import mybir
    from concourse.masks import make_identity

    f32 = mybir.dt.float32
    f32r = mybir.dt.float32r
    bf16 = mybir.dt.bfloat16
    AF = mybir.ActivationFunctionType
    AX = mybir.AxisListType
    MUL = mybir.AluOpType.mult

    nc = bacc.Bacc("TRN2", target_bir_lowering=False, debug=False)

    xT = nc.dram_tensor("xT", [KT, 128, BS], bf16, kind="ExternalInput")
    Wf = nc.dram_tensor("Wf", [KT, 128, NT], bf16, kind="ExternalInput")
    # leaf tables host-transposed to [d, (leaf, tree)] for contiguous DMA
    gt = nc.dram_tensor("gt", [LEAF_DIMS, N_LEAVES * NUM_TREES], bf16, kind="ExternalInput")
    lwt = nc.dram_tensor("lwt", [LEAF_DIMS, N_LEAVES * NUM_TREES], bf16, kind="ExternalInput")
    if use_bias:
        bias = nc.dram_tensor("bias", [1, NT], f32r, kind="ExternalInput")
    outT = nc.dram_tensor("outT", [LEAF_DIMS, BS], f32, kind="ExternalOutput")

    with tile.TileContext(nc) as tc:
        with tc.tile_pool(name="const", bufs=1) as cpool, \
             tc.tile_pool(name="wts", bufs=1) as wpool, \
             tc.tile_pool(name="psA", bufs=4, space="PSUM") as psA, \
             tc.tile_pool(name="psT", bufs=3, space="PSUM") as psT, \
             tc.tile_pool(name="psO", bufs=1, space="PSUM") as psO:

            ident = cpool.tile([128, 128], bf16, tag="ident")
            make_identity(nc, ident[:])

            # ---- input DMAs.  x + leaf tables on the ACT HW-DGE ring, the
            # (bigger) weight matrix on the SP ring, so they overlap. ----
            xk = []
            for k in range(KT):
                t = wpool.tile([128, BS], bf16, tag=f"xk{k}", name=f"xk{k}")
                nc.scalar.dma_start(t[:], xT[k, :, :])
                xk.append(t)

            wk = [wpool.tile([128, NT], bf16, tag=f"wk{k}", name=f"wk{k}") for k in range(KT)]
            for m in range(NCHUNK // 2):
                for k in range(KT):
                    nc.sync.dma_start(
                        wk[k][:, m * 2 * CHUNK:(m + 1) * 2 * CHUNK],
                        Wf[k, :, m * 2 * CHUNK:(m + 1) * 2 * CHUNK],
                    )
            if use_bias:
                bias_sb = cpool.tile([1, NT], f32r, tag="bias")
                nc.sync.dma_start(bias_sb[:], bias[:, :])
                ones1 = cpool.tile([1, 128], f32r, tag="ones1")
                nc.vector.memset(ones1[:], 1.0)

            wsm = wpool.tile([128, N_LEAVES, NUM_TREES], bf16, tag="wsm")
            gtile = wpool.tile([128, N_LEAVES, NUM_TREES], bf16, tag="gtile")
            nc.scalar.dma_start(gtile[:], gt[:, :].rearrange("d (l t) -> d l t", t=NUM_TREES))
            lwtile = wpool.tile([128, N_LEAVES, NUM_TREES], bf16, tag="lwtile")
            nc.scalar.dma_start(lwtile[:], lwt[:, :].rearrange("d (l t) -> d l t", t=NUM_TREES))

            def emit_phase0_compute():
                # w = leaf_weight * softmax(gates): exp on ACT, reduce on DVE,
                # normalize on the idle Pool engine (PE transposes deferred)
                nc.scalar.activation(gtile[:], gtile[:], AF.Exp)
                s = cpool.tile([128, N_LEAVES], f32, tag="s")
                nc.vector.reduce_sum(s[:], gtile[:], axis=AX.X)
                r = cpool.tile([128, N_LEAVES], f32, tag="r")
                nc.vector.reciprocal(r[:], s[:])
                for l in range(N_LEAVES):
                    nc.vector.scalar_tensor_tensor(
                        wsm[:, l, :], gtile[:, l, :], r[:, l:l + 1], lwtile[:, l, :],
                        op0=MUL, op1=MUL,
                    )

            # ---- main loop: batch-tile pairs ----
            with tc.tile_pool(name="gp", bufs=1) as gpool, \
                 tc.tile_pool(name="pp", bufs=2) as ppool, \
                 tc.tile_pool(name="pfp", bufs=2) as pfpool, \
                 tc.tile_pool(name="ptp", bufs=1) as pTpool, \
                 tc.tile_pool(name="outp", bufs=1) as outpool:

                out_ps = psO.tile([LEAF_DIMS, BS], f32, tag="out_ps")
                out_sb = outpool.tile([LEAF_DIMS, BS], f32, tag="out_sb")

                def emit_phaseD_slice(i):
                    bsl = slice(i * 128, (i + 1) * 128)
                    for j in range(NPT):
                        nc.tensor.matmul(out_ps[:, bsl],
                                         wT_all[:, j // 4, (j % 4) * 128:(j % 4 + 1) * 128],
                                         pT_all[:, j, bsl],
                                         start=(j == 0), stop=(j == NPT - 1))
                    nc.vector.tensor_copy(out_sb[:, bsl], out_ps[:, bsl])
                    nc.scalar.dma_start(outT[:, bsl], out_sb[:, bsl])

                # pT_all[:, j, :] = chunk j of p_T, [(leaf,tree)(128), b(512)]
                pT_all = pTpool.tile([128, NPT, BS], bf16, tag="pT")
                # g_all[:, i, :] = sigmoid gates for batch tile i
                g_all = gpool.tile([128, NBT, NT], bf16, tag="g")
                wT_all = wpool.tile([128, NPT // 4, 512], bf16, tag="wT")

                for pair in range(NBT // 2):
                    i0 = 2 * pair
                    # phase A: one 2-bank PSUM tile serves both batch tiles of
                    # the pair; chunks consumed in DMA arrival order
                    for n in range(NCHUNK):
                        csl = slice(n * CHUNK, (n + 1) * CHUNK)
                        for q in range(2):
                            bsl = slice((i0 + q) * 128, (i0 + q + 1) * 128)
                            lg = psA.tile([128, CHUNK], f32, tag="lg")
                            for k in range(KT):
                                nc.tensor.matmul(
                                    lg[:], xk[k][:, bsl], wk[k][:, csl],
                                    start=(k == 0),
                                    stop=(k == KT - 1 and not use_bias),
                                )
                            if use_bias:
                                nc.tensor.matmul(
                                    lg[:], ones1[:], bias_sb[:, csl],
                                    start=False, stop=True,
                                )
                            nc.scalar.activation(g_all[:, i0 + q, csl], lg[:], AF.Sigmoid)

                    if pair == 1:
                        # phase 0 PE part here: wsm is ready by now and this
                        # keeps the post-loop PE tail short
                        for jj in range(NPT // 4):
                            tp4 = psT.tile([128, 512], bf16, tag="tp")
                            for qq in range(4):
                                j = 4 * jj + qq
                                nc.tensor.transpose(
                                    tp4[:, qq * 128:(qq + 1) * 128],
                                    wsm[:, 2 * j:2 * j + 2, :], ident[:])
                            nc.scalar.copy(wT_all[:, jj, :], tp4[:])

                        for ii in range(2):
                            emit_phaseD_slice(ii)

                    for q in range(2):
                        i = i0 + q
                        bsl = slice(i * 128, (i + 1) * 128)
                        # phase B: block-layout level doubling (DVE, 2x mode)
                        pa = ppool.tile([128, 2048], bf16, tag="pa")
                        pb = ppool.tile([128, 2048], bf16, tag="pb")
                        pf = pfpool.tile([128, 4096], bf16, tag="pf")
                        # level 0: p = [g0 | 1-g0]
                        nc.vector.tensor_copy(pa[:, 0:64], g_all[:, i, 0:64])
                        nc.scalar.activation(pa[:, 64:128], g_all[:, i, 0:64],
                                             AF.Copy, bias=1.0, scale=-1.0)
                        cur, other = pa, pb
                        for lvl in range(1, MAX_DEPTH):
                            h = (2 ** lvl) * 64
                            off = (2 ** lvl - 1) * 64
                            dst = pf if lvl == MAX_DEPTH - 1 else other
                            nc.vector.tensor_mul(dst[:, 0:h], cur[:, 0:h],
                                                 g_all[:, i, off:off + h])
                            nc.vector.tensor_sub(dst[:, h:2 * h], cur[:, 0:h],
                                                 dst[:, 0:h])
                            cur, other = dst, cur

                        # phase C: transpose p -> [(leaf,tree), b] bf16;
                        # 4 chunks share one PSUM bank, one DVE copy per bank
                        for jj in range(NPT // 4):
                            tp4 = psT.tile([128, 512], bf16, tag="tp")
                            for qq in range(4):
                                j = 4 * jj + qq
                                nc.tensor.transpose(
                                    tp4[:, qq * 128:(qq + 1) * 128],
                                    pf[:, j * 128:(j + 1) * 128], ident[:])
                            nc.vector.tensor_copy(
                                pT_all[:, 4 * jj:4 * jj + 4, bsl], tp4[:])

                        if pair == 1:
                            emit_phaseD_slice(i)

                    if pair == 0:
                        emit_phase0_compute()



    nc.finalize()
    return nc


def _get_nc(use_bias: bool):
    if use_bias not in _BUILT:
        _BUILT[use_bias] = _build(use_bias)
    return _BUILT[use_bias]


def _make_in_maps(x, W, b, leaf_weight, gates):
    x = np.ascontiguousarray(np.asarray(x, dtype=np.float32))
    W = np.asarray(W, dtype=np.float32)
    b = np.asarray(b, dtype=np.float32)
    leaf_weight = np.asarray(leaf_weight, dtype=np.float32)
    gates = np.asarray(gates, dtype=np.float32)

    use_bias = bool(np.any(b))
    # host-side layout prep (permutations / transposes / bf16 cast)
    Wp = W[_NODES_PERM]                                   # [63, 512, 64]
    Wf = np.ascontiguousarray(
        Wp.transpose(1, 0, 2).reshape(KT, 128, NT).astype(BF16))
    # leaf tables -> [d, (leaf, tree)] so the DMA is contiguous per partition
    gt = np.ascontiguousarray(
        gates[_LEAF_PERM].transpose(1, 0, 2).reshape(LEAF_DIMS, -1).astype(BF16))
    lwt = np.ascontiguousarray(
        leaf_weight[_LEAF_PERM].transpose(1, 0, 2).reshape(LEAF_DIMS, -1).astype(BF16))
    if use_bias:
        bias = np.ascontiguousarray(b[_NODES_PERM].reshape(1, NT))

    in_maps = []
    for c in range(N_CORES):
        xs = x[c * BS:(c + 1) * BS]                       # [512, 512]
        xTc = np.ascontiguousarray(xs.T.reshape(KT, 128, BS).astype(BF16))
        m = {"xT": xTc, "Wf": Wf, "gt": gt, "lwt": lwt}
        if use_bias:
            m["bias"] = bias
        in_maps.append(m)
    return use_bias, in_maps


def kernel(x, W, b, leaf_weight, gates):
    from concourse.bass_utils import run_bass_kernel_spmd

    use_bias, in_maps = _make_in_maps(x, W, b, leaf_weight, gates)
    nc = _get_nc(use_bias)

    res = run_bass_kernel_spmd(nc, in_maps, core_ids=list(range(N_CORES)))
    out = np.empty((BATCH, LEAF_DIMS), dtype=np.float32)
    for c in range(N_CORES):
        out[c * BS:(c + 1) * BS] = res.results[c]["outT"].T
    return out


# revision 34
# speedup vs baseline: 139.7910x; 139.7910x over previous
"""MixtureOfExpertsTreeEnsemble Trainium2 kernel (8-core SPMD, batch data-parallel).

Math (per batch row b, tree t):
  g[b,n,t] = sigmoid(x[b] @ W[n,:,t] + bias[n,t])          63 internal nodes
  p[b,l,t] = prod of g / (1-g) along root->leaf path        64 leaves
  w[l,d,t] = leaf_weight[l,d,t] * softmax_t(gates[l,d,t])
  out[b,d] = sum_{l,t} p[b,l,t] * w[l,d,t]

Sharding: batch 4096 -> 8 cores x 512 rows; node weights / leaf tables are
replicated (small).  No collectives; host concatenates the per-core outputs.

Device-side structure (per core):
  * all streamed operands are bf16 (the gates are bf16 downstream anyway, so
    f32 logits precision would be wasted); PSUM accumulation stays f32
  * phase A (PE):   logits as [b_tile(128), (node,tree)] bf16 matmuls.
    Loop is (btile-pair, chunk): each (node,tree) chunk is consumed as soon
    as its DMA lands, and one [128,1008] 2-bank PSUM tile serves two batch
    tiles -> half the sigmoid instructions on ACT
  * phase B (DVE):  leaf path probabilities by level doubling in a *block*
    layout (children stored [left | right]) so every op is contiguous and
    bf16 (DVE 2x mode); host pre-permutes node order (bit-reversal within
    each level) and the leaf tables to match
  * phase C (PE):   p transposed to [(leaf,tree), b] bf16 chunks, 4 chunks
    per PSUM bank, one DVE copy per bank
  * phase 0:        w = leaf_weight * softmax(gates) with (l,d) on partitions
    and t free: exp on ACT, reduce on DVE, the 64 normalize ops on the idle
    Pool engine; PE transposes are emitted after the batch loop so they
    never block phase A on the leaf-table DMAs
  * phase D (PE):   out_T[d,b] = sum_chunks w_T.T @ p_T, host transposes back
  * DMA: weight matrix on the SP HW-DGE ring; x / leaf tables / output on the
    ACT ring so the two streams overlap
"""

import sys

sys.path.insert(0, "/opt/trn_rl_repo")

import ml_dtypes
import numpy as np

BF16 = np.dtype(ml_dtypes.bfloat16)

MAX_DEPTH = 6
NUM_TREES = 64
LEAF_DIMS = 128
D_IN = 512
BATCH = 4096
N_INTERNAL = 63
N_LEAVES = 64
N_CORES = 8
BS = BATCH // N_CORES          # 512 batch rows per core
KT = D_IN // 128               # 4 contraction tiles
NT = N_INTERNAL * NUM_TREES    # 4032 (node,tree) pairs
NCHUNK = 8
CHUNK = NT // NCHUNK           # 504
NBT = BS // 128                # 4 batch tiles per core
NPT = N_LEAVES * NUM_TREES // 128  # 32 transpose chunks of (leaf,tree)


def _bitrev(x: int, bits: int) -> int:
    r = 0
    for _ in range(bits):
        r = (r << 1) | (x & 1)
        x >>= 1
    return r


# block-recursion orderings (see module docstring)
_NODES_PERM = np.array(
    [(2**lvl - 1) + _bitrev(j, lvl) for lvl in range(MAX_DEPTH) for j in range(2**lvl)]
)
_LEAF_PERM = np.array([_bitrev(j, MAX_DEPTH) for j in range(N_LEAVES)])

_BUILT = {}


def _build(use_bias: bool):
    """Build + finalize the per-core Bass program."""
    import concourse.bacc as bacc
    import concourse.tile as tile
    from concourse import mybir
    from concourse.masks import make_identity

    f32 = mybir.dt.float32
    f32r = mybir.dt.float32r
    bf16 = mybir.dt.bfloat16
    AF = mybir.ActivationFunctionType
    AX = mybir.AxisListType
    MUL = mybir.AluOpType.mult

    nc = bacc.Bacc("TRN2", target_bir_lowering=False, debug=False)

    xT = nc.dram_tensor("xT", [KT, 128, BS], bf16, kind="ExternalInput")
    Wf = nc.dram_tensor("Wf", [KT, 128, NT], bf16, kind="ExternalInput")
    # leaf tables host-transposed to [d, (leaf, tree)] for contiguous DMA
    gt = nc.dram_tensor("gt", [LEAF_DIMS, N_LEAVES * NUM_TREES], bf16, kind="ExternalInput")
    lwt = nc.dram_tensor("lwt", [LEAF_DIMS, N_LEAVES * NUM_TREES], bf16, kind="ExternalInput")
    if use_bias:
        bias = nc.dram_tensor("bias", [1, NT], bf16, kind="ExternalInput")
    outT = nc.dram_tensor("outT", [LEAF_DIMS, BS], f32, kind="ExternalOutput")

    with tile.TileContext(nc) as tc:
        with tc.tile_pool(name="const", bufs=1) as cpool, \
             tc.tile_pool(name="wts", bufs=1) as wpool, \
             tc.tile_pool(name="psA", bufs=4, space="PSUM") as psA, \
             tc.tile_pool(name="psT", bufs=3, space="PSUM") as psT, \
             tc.tile_pool(name="psO", bufs=1, space="PSUM") as psO:

            ident = cpool.tile([128, 128], bf16, tag="ident")
            make_identity(nc, ident[:])

            # ---- input DMAs.  x + leaf tables on the ACT HW-DGE ring, the
            # (bigger) weight matrix on the SP ring, so they overlap. ----
            xk = []
            for k in range(KT):
                t = wpool.tile([128, BS], bf16, tag=f"xk{k}", name=f"xk{k}")
                nc.scalar.dma_start(t[:], xT[k, :, :])
                xk.append(t)

            wk = [wpool.tile([128, NT], bf16, tag=f"wk{k}", name=f"wk{k}") for k in range(KT)]
            for m in range(NCHUNK // 2):
                for k in range(KT):
                    nc.sync.dma_start(
                        wk[k][:, m * 2 * CHUNK:(m + 1) * 2 * CHUNK],
                        Wf[k, :, m * 2 * CHUNK:(m + 1) * 2 * CHUNK],
                    )
            if use_bias:
                bias_sb = cpool.tile([1, NT], bf16, tag="bias")
                nc.sync.dma_start(bias_sb[:], bias[:, :])
                ones1 = cpool.tile([1, 128], bf16, tag="ones1")
                nc.gpsimd.memset(ones1[:], 1.0)

            wsm = wpool.tile([128, N_LEAVES, NUM_TREES], bf16, tag="wsm")
            gtile = wpool.tile([128, N_LEAVES, NUM_TREES], bf16, tag="gtile")
            nc.sync.dma_start(gtile[:], gt[:, :].rearrange("d (l t) -> d l t", t=NUM_TREES))
            lwtile = wpool.tile([128, N_LEAVES, NUM_TREES], bf16, tag="lwtile")
            nc.sync.dma_start(lwtile[:], lwt[:, :].rearrange("d (l t) -> d l t", t=NUM_TREES))

            def emit_phase0_compute():
                # w = leaf_weight * softmax(gates): exp on ACT, reduce on DVE,
                # normalize on the idle Pool engine (PE transposes deferred)
                nc.scalar.activation(gtile[:], gtile[:], AF.Exp)
                s = cpool.tile([128, N_LEAVES], f32, tag="s")
                nc.vector.reduce_sum(s[:], gtile[:], axis=AX.X)
                r = cpool.tile([128, N_LEAVES], f32, tag="r")
                nc.vector.reciprocal(r[:], s[:])
                for l in range(N_LEAVES):
                    nc.vector.scalar_tensor_tensor(
                        wsm[:, l, :], gtile[:, l, :], r[:, l:l + 1], lwtile[:, l, :],
                        op0=MUL, op1=MUL,
                    )

            # ---- main loop: batch-tile pairs ----
            with tc.tile_pool(name="gp", bufs=1) as gpool, \
                 tc.tile_pool(name="pp", bufs=2) as ppool, \
                 tc.tile_pool(name="pfp", bufs=2) as pfpool, \
                 tc.tile_pool(name="ptp", bufs=1) as pTpool, \
                 tc.tile_pool(name="outp", bufs=1) as outpool:

                out_ps = psO.tile([LEAF_DIMS, BS], f32, tag="out_ps")
                out_sb = outpool.tile([LEAF_DIMS, BS], f32, tag="out_sb")

                def emit_phaseD_slice(i):
                    bsl = slice(i * 128, (i + 1) * 128)
                    for j in range(NPT):
                        nc.tensor.matmul(out_ps[:, bsl],
                                         wT_all[:, j // 4, (j % 4) * 128:(j % 4 + 1) * 128],
                                         pT_all[:, j, bsl],
                                         start=(j == 0), stop=(j == NPT - 1))
                    nc.vector.tensor_copy(out_sb[:, bsl], out_ps[:, bsl])
                    nc.scalar.dma_start(outT[:, bsl], out_sb[:, bsl])

                # pT_all[:, j, :] = chunk j of p_T, [(leaf,tree)(128), b(512)]
                pT_all = pTpool.tile([128, NPT, BS], bf16, tag="pT")
                # g_all[:, i, :] = sigmoid gates for batch tile i
                g_all = gpool.tile([128, NBT, NT], bf16, tag="g")
                wT_all = wpool.tile([128, NPT // 4, 512], bf16, tag="wT")

                for pair in range(NBT // 2):
                    i0 = 2 * pair
                    # phase A: one 2-bank PSUM tile serves both batch tiles of
                    # the pair; chunks consumed in DMA arrival order
                    for n in range(NCHUNK):
                        csl = slice(n * CHUNK, (n + 1) * CHUNK)
                        for q in range(2):
                            bsl = slice((i0 + q) * 128, (i0 + q + 1) * 128)
                            lg = psA.tile([128, CHUNK], f32, tag="lg")
                            for k in range(KT):
                                nc.tensor.matmul(
                                    lg[:], xk[k][:, bsl], wk[k][:, csl],
                                    start=(k == 0),
                                    stop=(k == KT - 1 and not use_bias),
                                )
                            if use_bias:
                                nc.tensor.matmul(
                                    lg[:], ones1[:], bias_sb[:, csl],
                                    start=False, stop=True,
                                )
                            nc.scalar.activation(g_all[:, i0 + q, csl], lg[:], AF.Sigmoid)

                    if pair == 1:
                        # phase 0 PE part here: wsm is ready by now and this
                        # keeps the post-loop PE tail short
                        for jj in range(NPT // 4):
                            tp4 = psT.tile([128, 512], bf16, tag="tp")
                            for qq in range(4):
                                j = 4 * jj + qq
                                nc.tensor.transpose(
                                    tp4[:, qq * 128:(qq + 1) * 128],
                                    wsm[:, 2 * j:2 * j + 2, :], ident[:])
                            nc.scalar.copy(wT_all[:, jj, :], tp4[:])

                        for ii in range(2):
                            emit_phaseD_slice(ii)

                    for q in range(2):
                        i = i0 + q
                        bsl = slice(i * 128, (i + 1) * 128)
                        # phase B: block-layout level doubling (DVE, 2x mode)
                        pa = ppool.tile([128, 2048], bf16, tag="pa")
                        pb = ppool.tile([128, 2048], bf16, tag="pb")
                        pf = pfpool.tile([128, 4096], bf16, tag="pf")
                        # level 0: p = [g0 | 1-g0]
                        nc.vector.tensor_copy(pa[:, 0:64], g_all[:, i, 0:64])
                        nc.scalar.activation(pa[:, 64:128], g_all[:, i, 0:64],
                                             AF.Copy, bias=1.0, scale=-1.0)
                        cur, other = pa, pb
                        for lvl in range(1, MAX_DEPTH):
                            h = (2 ** lvl) * 64
                            off = (2 ** lvl - 1) * 64
                            dst = pf if lvl == MAX_DEPTH - 1 else other
                            nc.vector.tensor_mul(dst[:, 0:h], cur[:, 0:h],
                                                 g_all[:, i, off:off + h])
                            nc.vector.tensor_sub(dst[:, h:2 * h], cur[:, 0:h],
                                                 dst[:, 0:h])
                            cur, other = dst, cur

                        # phase C: transpose p -> [(leaf,tree), b] bf16;
                        # 4 chunks share one PSUM bank, one DVE copy per bank
                        for jj in range(NPT // 4):
                            tp4 = psT.tile([128, 512], bf16, tag="tp")
                            for qq in range(4):
                                j = 4 * jj + qq
                                nc.tensor.transpose(
                                    tp4[:, qq * 128:(qq + 1) * 128],
                                    pf[:, j * 128:(j + 1) * 128], ident[:])
                            mod = 2 if pair == 1 else 3
                            if jj % mod == mod - 1:
                                nc.scalar.copy(
                                    pT_all[:, 4 * jj:4 * jj + 4, bsl], tp4[:])
                            else:
                                nc.vector.tensor_copy(
                                    pT_all[:, 4 * jj:4 * jj + 4, bsl], tp4[:])

                        if pair == 1:
                            emit_phaseD_slice(i)

                    if pair == 0:
                        emit_phase0_compute()



    nc.finalize()
    return nc


def _get_nc(use_bias: bool):
    if use_bias not in _BUILT:
        _BUILT[use_bias] = _build(use_bias)
    return _BUILT[use_bias]


def _make_in_maps(x, W, b, leaf_weight, gates):
    x = np.ascontiguousarray(np.asarray(x, dtype=np.float32))
    W = np.asarray(W, dtype=np.float32)
    b = np.asarray(b, dtype=np.float32)
    leaf_weight = np.asarray(leaf_weight, dtype=np.float32)
    gates = np.asarray(gates, dtype=np.float32)

    use_bias = bool(np.any(b))
    # host-side layout prep (permutations / transposes / bf16 cast)
    Wp = W[_NODES_PERM]                                   # [63, 512, 64]
    Wf = np.ascontiguousarray(
        Wp.transpose(1, 0, 2).reshape(KT, 128, NT).astype(BF16))
    # leaf tables -> [d, (leaf, tree)] so the DMA is contiguous per partition
    gt = np.ascontiguousarray(
        gates[_LEAF_PERM].transpose(1, 0, 2).reshape(LEAF_DIMS, -1).astype(BF16))
    lwt = np.ascontiguousarray(
        leaf_weight[_LEAF_PERM].transpose(1, 0, 2).reshape(LEAF_DIMS, -1).astype(BF16))
    if use_bias:
        bias = np.ascontiguousarray(b[_NODES_PERM].reshape(1, NT).astype(BF16))

    in_maps = []
    for c in range(N_CORES):
        xs = x[c * BS:(c + 1) * BS]                       # [512, 512]
        xTc = np.ascontiguousarray(xs.T.reshape(KT, 128, BS).astype(BF16))
        m = {"xT": xTc, "Wf": Wf, "gt": gt, "lwt": lwt}
        if use_bias:
            m["bias"] = bias
        in_maps.append(m)
    return use_bias, in_maps


def kernel(x, W, b, leaf_weight, gates):
    from concourse.bass_utils import run_bass_kernel_spmd

    use_bias, in_maps = _make_in_maps(x, W, b, leaf_weight, gates)
    nc = _get_nc(use_bias)

    res = run_bass_kernel_spmd(nc, in_maps, core_ids=list(range(N_CORES)))
    out = np.empty((BATCH, LEAF_DIMS), dtype=np.float32)
    for c in range(N_CORES):
        out[c * BS:(c + 1) * BS] = res.results[c]["outT"].T
    return out


# revision 42
# speedup vs baseline: 145.6067x; 1.0416x over previous
"""MixtureOfExpertsTreeEnsemble Trainium2 kernel (8-core SPMD, batch data-parallel).

Math (per batch row b, tree t):
  g[b,n,t] = sigmoid(x[b] @ W[n,:,t] + bias[n,t])          63 internal nodes
  p[b,l,t] = prod of g / (1-g) along root->leaf path        64 leaves
  w[l,d,t] = leaf_weight[l,d,t] * softmax_t(gates[l,d,t])
  out[b,d] = sum_{l,t} p[b,l,t] * w[l,d,t]

Sharding: batch 4096 -> 8 cores x 512 rows; node weights / leaf tables are
replicated (small).  No collectives; host concatenates the per-core outputs.

Device-side structure (per core):
  * all streamed operands are bf16 (the gates are bf16 downstream anyway, so
    f32 logits precision would be wasted); PSUM accumulation stays f32
  * phase A (PE):   logits as [b_tile(128), (node,tree)] bf16 matmuls.
    Loop is (btile-pair, chunk): each (node,tree) chunk is consumed as soon
    as its DMA lands, and one [128,1008] 2-bank PSUM tile serves two batch
    tiles -> half the sigmoid instructions on ACT
  * phase B (DVE):  leaf path probabilities by level doubling in a *block*
    layout (children stored [left | right]) so every op is contiguous and
    bf16 (DVE 2x mode); host pre-permutes node order (bit-reversal within
    each level) and the leaf tables to match
  * phase C (PE):   p transposed to [(leaf,tree), b] bf16 chunks, 4 chunks
    per PSUM bank, one DVE copy per bank
  * phase 0:        w = leaf_weight * softmax(gates) with (l,d) on partitions
    and t free: exp on ACT, reduce on DVE, the 64 normalize ops on the idle
    Pool engine; PE transposes are emitted after the batch loop so they
    never block phase A on the leaf-table DMAs
  * phase D (PE):   out_T[d,b] = sum_chunks w_T.T @ p_T, host transposes back
  * DMA: weight matrix on the SP HW-DGE ring; x / leaf tables / output on the
    ACT ring so the two streams overlap
"""

import sys

sys.path.insert(0, "/opt/trn_rl_repo")

import ml_dtypes
import numpy as np

BF16 = np.dtype(ml_dtypes.bfloat16)

MAX_DEPTH = 6
NUM_TREES = 64
LEAF_DIMS = 128
D_IN = 512
BATCH = 4096
N_INTERNAL = 63
N_LEAVES = 64
N_CORES = 8
BS = BATCH // N_CORES          # 512 batch rows per core
KT = D_IN // 128               # 4 contraction tiles
NT = N_INTERNAL * NUM_TREES    # 4032 (node,tree) pairs
NCHUNK = 8
CHUNK = NT // NCHUNK           # 504
NBT = BS // 128                # 4 batch tiles per core
NPT = N_LEAVES * NUM_TREES // 128  # 32 transpose chunks of (leaf,tree)


def _bitrev(x: int, bits: int) -> int:
    r = 0
    for _ in range(bits):
        r = (r << 1) | (x & 1)
        x >>= 1
    return r


# block-recursion orderings (see module docstring)
_NODES_PERM = np.array(
    [(2**lvl - 1) + _bitrev(j, lvl) for lvl in range(MAX_DEPTH) for j in range(2**lvl)]
)
_LEAF_PERM = np.array([_bitrev(j, MAX_DEPTH) for j in range(N_LEAVES)])

_BUILT = {}


def _build(use_bias: bool):
    """Build + finalize the per-core Bass program."""
    import concourse.bacc as bacc
    import concourse.tile as tile
    from concourse import mybir
    from concourse.masks import make_identity

    f32 = mybir.dt.float32
    f32r = mybir.dt.float32r
    bf16 = mybir.dt.bfloat16
    AF = mybir.ActivationFunctionType
    AX = mybir.AxisListType
    MUL = mybir.AluOpType.mult

    nc = bacc.Bacc("TRN2", target_bir_lowering=False, debug=False)

    xT = nc.dram_tensor("xT", [KT, 128, BS], bf16, kind="ExternalInput")
    Wf = nc.dram_tensor("Wf", [KT, 128, NT], bf16, kind="ExternalInput")
    # leaf tables host-transposed to [d, (leaf, tree)] for contiguous DMA
    gt = nc.dram_tensor("gt", [LEAF_DIMS, N_LEAVES * NUM_TREES], bf16, kind="ExternalInput")
    lwt = nc.dram_tensor("lwt", [LEAF_DIMS, N_LEAVES * NUM_TREES], bf16, kind="ExternalInput")
    if use_bias:
        bias = nc.dram_tensor("bias", [1, NT], bf16, kind="ExternalInput")
    outT = nc.dram_tensor("outT", [LEAF_DIMS, BS], f32, kind="ExternalOutput")

    with tile.TileContext(nc) as tc:
        with tc.tile_pool(name="const", bufs=1) as cpool, \
             tc.tile_pool(name="wts", bufs=1) as wpool, \
             tc.tile_pool(name="psA", bufs=4, space="PSUM") as psA, \
             tc.tile_pool(name="psT", bufs=3, space="PSUM") as psT, \
             tc.tile_pool(name="psO", bufs=1, space="PSUM") as psO:

            ident = cpool.tile([128, 128], bf16, tag="ident")
            make_identity(nc, ident[:])

            # ---- input DMAs.  x + leaf tables on the ACT HW-DGE ring, the
            # (bigger) weight matrix on the SP ring, so they overlap. ----
            xk = []
            for k in range(KT):
                t = wpool.tile([128, BS], bf16, tag=f"xk{k}", name=f"xk{k}")
                nc.scalar.dma_start(t[:], xT[k, :, :])
                xk.append(t)

            wk = [wpool.tile([128, NT], bf16, tag=f"wk{k}", name=f"wk{k}") for k in range(KT)]
            for m in range(NCHUNK // 2):
                for k in range(KT):
                    nc.sync.dma_start(
                        wk[k][:, m * 2 * CHUNK:(m + 1) * 2 * CHUNK],
                        Wf[k, :, m * 2 * CHUNK:(m + 1) * 2 * CHUNK],
                    )
            if use_bias:
                bias_sb = cpool.tile([1, NT], bf16, tag="bias")
                nc.sync.dma_start(bias_sb[:], bias[:, :])
                ones1 = cpool.tile([1, 128], bf16, tag="ones1")
                nc.gpsimd.memset(ones1[:], 1.0)

            wsm = wpool.tile([128, N_LEAVES, NUM_TREES], bf16, tag="wsm")
            gtile = wpool.tile([128, N_LEAVES, NUM_TREES], bf16, tag="gtile")
            nc.sync.dma_start(gtile[:], gt[:, :].rearrange("d (l t) -> d l t", t=NUM_TREES))
            lwtile = wpool.tile([128, N_LEAVES, NUM_TREES], bf16, tag="lwtile")
            nc.sync.dma_start(lwtile[:], lwt[:, :].rearrange("d (l t) -> d l t", t=NUM_TREES))

            def emit_phase0_compute():
                # w = leaf_weight * softmax(gates): exp on ACT, reduce on DVE,
                # normalize on the idle Pool engine (PE transposes deferred)
                nc.scalar.activation(gtile[:], gtile[:], AF.Exp)
                ehalf = cpool.tile([128, N_LEAVES, NUM_TREES // 2], bf16, tag="ehalf")
                nc.vector.tensor_add(ehalf[:], gtile[:, :, 0:NUM_TREES // 2],
                                     gtile[:, :, NUM_TREES // 2:NUM_TREES])
                s = cpool.tile([128, N_LEAVES], f32, tag="s")
                nc.vector.reduce_sum(s[:], ehalf[:], axis=AX.X)
                r = cpool.tile([128, N_LEAVES], f32, tag="r")
                nc.vector.reciprocal(r[:], s[:])
                for l in range(N_LEAVES):
                    nc.vector.scalar_tensor_tensor(
                        wsm[:, l, :], gtile[:, l, :], r[:, l:l + 1], lwtile[:, l, :],
                        op0=MUL, op1=MUL,
                    )

            # ---- main loop: batch-tile pairs ----
            with tc.tile_pool(name="gp", bufs=1) as gpool, \
                 tc.tile_pool(name="pp", bufs=2) as ppool, \
                 tc.tile_pool(name="pfp", bufs=2) as pfpool, \
                 tc.tile_pool(name="ptp", bufs=1) as pTpool, \
                 tc.tile_pool(name="outp", bufs=1) as outpool:

                out_ps = psO.tile([LEAF_DIMS, BS], f32, tag="out_ps")
                out_sb = outpool.tile([LEAF_DIMS, BS], f32, tag="out_sb")

                def emit_phaseD_slice(i, width=1):
                    bsl = slice(i * 128, (i + width) * 128)
                    for j in range(NPT):
                        nc.tensor.matmul(out_ps[:, bsl],
                                         wT_all[:, j // 4, (j % 4) * 128:(j % 4 + 1) * 128],
                                         pT_all[:, j, bsl],
                                         start=(j == 0), stop=(j == NPT - 1))
                    nc.vector.tensor_copy(out_sb[:, bsl], out_ps[:, bsl])
                    nc.scalar.dma_start(outT[:, bsl], out_sb[:, bsl])

                # pT_all[:, j, :] = chunk j of p_T, [(leaf,tree)(128), b(512)]
                pT_all = pTpool.tile([128, NPT, BS], bf16, tag="pT")
                # g_all[:, i, :] = sigmoid gates for batch tile i
                g_all = gpool.tile([128, NBT, NT], bf16, tag="g")
                wT_all = wpool.tile([128, NPT // 4, 512], bf16, tag="wT")

                for pair in range(NBT // 2):
                    i0 = 2 * pair
                    # phase A: one 2-bank PSUM tile serves both batch tiles of
                    # the pair; chunks consumed in DMA arrival order
                    for q in range(2):
                        bsl = slice((i0 + q) * 128, (i0 + q + 1) * 128)
                        for n in range(NCHUNK):
                            csl = slice(n * CHUNK, (n + 1) * CHUNK)
                            lg = psA.tile([128, CHUNK], f32, tag="lg")
                            for k in range(KT):
                                nc.tensor.matmul(
                                    lg[:], xk[k][:, bsl], wk[k][:, csl],
                                    start=(k == 0),
                                    stop=(k == KT - 1 and not use_bias),
                                )
                            if use_bias:
                                nc.tensor.matmul(
                                    lg[:], ones1[:], bias_sb[:, csl],
                                    start=False, stop=True,
                                )
                            nc.scalar.activation(g_all[:, i0 + q, csl], lg[:], AF.Sigmoid)

                    if pair == 1:
                        # phase 0 PE part here: wsm is ready by now and this
                        # keeps the post-loop PE tail short
                        for jj in range(NPT // 4):
                            tp4 = psT.tile([128, 512], bf16, tag="tp")
                            for qq in range(4):
                                j = 4 * jj + qq
                                nc.tensor.transpose(
                                    tp4[:, qq * 128:(qq + 1) * 128],
                                    wsm[:, 2 * j:2 * j + 2, :], ident[:])
                            nc.scalar.copy(wT_all[:, jj, :], tp4[:])

                        emit_phaseD_slice(0, width=2)

                    for q in range(2):
                        i = i0 + q
                        bsl = slice(i * 128, (i + 1) * 128)
                        # phase B: block-layout level doubling (DVE, 2x mode)
                        pa = ppool.tile([128, 2048], bf16, tag="pa")
                        pb = ppool.tile([128, 2048], bf16, tag="pb")
                        pf = pfpool.tile([128, 4096], bf16, tag="pf")
                        # level 0: p = [g0 | 1-g0]
                        nc.vector.tensor_copy(pa[:, 0:64], g_all[:, i, 0:64])
                        nc.scalar.activation(pa[:, 64:128], g_all[:, i, 0:64],
                                             AF.Copy, bias=1.0, scale=-1.0)
                        cur, other = pa, pb
                        for lvl in range(1, MAX_DEPTH):
                            h = (2 ** lvl) * 64
                            off = (2 ** lvl - 1) * 64
                            dst = pf if lvl == MAX_DEPTH - 1 else other
                            nc.vector.tensor_mul(dst[:, 0:h], cur[:, 0:h],
                                                 g_all[:, i, off:off + h])
                            nc.vector.tensor_sub(dst[:, h:2 * h], cur[:, 0:h],
                                                 dst[:, 0:h])
                            cur, other = dst, cur

                        # phase C: transpose p -> [(leaf,tree), b] bf16;
                        # 4 chunks share one PSUM bank, one DVE copy per bank
                        for jj in range(NPT // 4):
                            tp4 = psT.tile([128, 512], bf16, tag="tp")
                            for qq in range(4):
                                j = 4 * jj + qq
                                nc.tensor.transpose(
                                    tp4[:, qq * 128:(qq + 1) * 128],
                                    pf[:, j * 128:(j + 1) * 128], ident[:])
                            mod = 2 if pair == 1 else 3
                            if jj % mod == mod - 1:
                                nc.scalar.copy(
                                    pT_all[:, 4 * jj:4 * jj + 4, bsl], tp4[:])
                            else:
                                nc.vector.tensor_copy(
                                    pT_all[:, 4 * jj:4 * jj + 4, bsl], tp4[:])

                        if pair == 1:
                            emit_phaseD_slice(i)

                    if pair == 0:
                        emit_phase0_compute()



    nc.finalize()
    return nc


def _get_nc(use_bias: bool):
    if use_bias not in _BUILT:
        _BUILT[use_bias] = _build(use_bias)
    return _BUILT[use_bias]


def _make_in_maps(x, W, b, leaf_weight, gates):
    x = np.ascontiguousarray(np.asarray(x, dtype=np.float32))
    W = np.asarray(W, dtype=np.float32)
    b = np.asarray(b, dtype=np.float32)
    leaf_weight = np.asarray(leaf_weight, dtype=np.float32)
    gates = np.asarray(gates, dtype=np.float32)

    use_bias = bool(np.any(b))
    # host-side layout prep (permutations / transposes / bf16 cast)
    Wp = W[_NODES_PERM]                                   # [63, 512, 64]
    Wf = np.ascontiguousarray(
        Wp.transpose(1, 0, 2).reshape(KT, 128, NT).astype(BF16))
    # leaf tables -> [d, (leaf, tree)] so the DMA is contiguous per partition
    gt = np.ascontiguousarray(
        gates[_LEAF_PERM].transpose(1, 0, 2).reshape(LEAF_DIMS, -1).astype(BF16))
    lwt = np.ascontiguousarray(
        leaf_weight[_LEAF_PERM].transpose(1, 0, 2).reshape(LEAF_DIMS, -1).astype(BF16))
    if use_bias:
        bias = np.ascontiguousarray(b[_NODES_PERM].reshape(1, NT).astype(BF16))

    in_maps = []
    for c in range(N_CORES):
        xs = x[c * BS:(c + 1) * BS]                       # [512, 512]
        xTc = np.ascontiguousarray(xs.T.reshape(KT, 128, BS).astype(BF16))
        m = {"xT": xTc, "Wf": Wf, "gt": gt, "lwt": lwt}
        if use_bias:
            m["bias"] = bias
        in_maps.append(m)
    return use_bias, in_maps


def kernel(x, W, b, leaf_weight, gates):
    from concourse.bass_utils import run_bass_kernel_spmd

    use_bias, in_maps = _make_in_maps(x, W, b, leaf_weight, gates)
    nc = _get_nc(use_bias)

    res = run_bass_kernel_spmd(nc, in_maps, core_ids=list(range(N_CORES)))
    out = np.empty((BATCH, LEAF_DIMS), dtype=np.float32)
    for c in range(N_CORES):
        out[c * BS:(c + 1) * BS] = res.results[c]["outT"].T
    return out


# revision 50
# speedup vs baseline: 147.4479x; 1.0126x over previous
"""MixtureOfExpertsTreeEnsemble Trainium2 kernel (8-core SPMD, batch data-parallel).

Math (per batch row b, tree t):
  g[b,n,t] = sigmoid(x[b] @ W[n,:,t] + bias[n,t])          63 internal nodes
  p[b,l,t] = prod of g / (1-g) along root->leaf path        64 leaves
  w[l,d,t] = leaf_weight[l,d,t] * softmax_t(gates[l,d,t])
  out[b,d] = sum_{l,t} p[b,l,t] * w[l,d,t]

Sharding: batch 4096 -> 8 cores x 512 rows; node weights / leaf tables are
replicated (small).  No collectives; host concatenates the per-core outputs.

Device-side structure (per core):
  * all streamed operands are bf16 (the gates are bf16 downstream anyway, so
    f32 logits precision would be wasted); PSUM accumulation stays f32
  * phase A (PE):   logits as [b_tile(128), (node,tree)] bf16 matmuls.
    Loop is (btile-pair, chunk): each (node,tree) chunk is consumed as soon
    as its DMA lands, and one [128,1008] 2-bank PSUM tile serves two batch
    tiles -> half the sigmoid instructions on ACT
  * phase B (DVE):  leaf path probabilities by level doubling in a *block*
    layout (children stored [left | right]) so every op is contiguous and
    bf16 (DVE 2x mode); host pre-permutes node order (bit-reversal within
    each level) and the leaf tables to match
  * phase C (PE):   p transposed to [(leaf,tree), b] bf16 chunks, 4 chunks
    per PSUM bank, one DVE copy per bank
  * phase 0:        w = leaf_weight * softmax(gates) with (l,d) on partitions
    and t free: exp on ACT, reduce on DVE, the 64 normalize ops on the idle
    Pool engine; PE transposes are emitted after the batch loop so they
    never block phase A on the leaf-table DMAs
  * phase D (PE):   out_T[d,b] = sum_chunks w_T.T @ p_T, host transposes back
  * DMA: weight matrix on the SP HW-DGE ring; x / leaf tables / output on the
    ACT ring so the two streams overlap
"""

import sys

sys.path.insert(0, "/opt/trn_rl_repo")

import ml_dtypes
import numpy as np

BF16 = np.dtype(ml_dtypes.bfloat16)

MAX_DEPTH = 6
NUM_TREES = 64
LEAF_DIMS = 128
D_IN = 512
BATCH = 4096
N_INTERNAL = 63
N_LEAVES = 64
N_CORES = 8
BS = BATCH // N_CORES          # 512 batch rows per core
KT = D_IN // 128               # 4 contraction tiles
NT = N_INTERNAL * NUM_TREES    # 4032 (node,tree) pairs
NCHUNK = 8
CHUNK = NT // NCHUNK           # 504
NBT = BS // 128                # 4 batch tiles per core
NPT = N_LEAVES * NUM_TREES // 128  # 32 transpose chunks of (leaf,tree)


def _bitrev(x: int, bits: int) -> int:
    r = 0
    for _ in range(bits):
        r = (r << 1) | (x & 1)
        x >>= 1
    return r


# block-recursion orderings (see module docstring)
_NODES_PERM = np.array(
    [(2**lvl - 1) + _bitrev(j, lvl) for lvl in range(MAX_DEPTH) for j in range(2**lvl)]
)
_LEAF_PERM = np.array([_bitrev(j, MAX_DEPTH) for j in range(N_LEAVES)])

_BUILT = {}


def _build(use_bias: bool):
    """Build + finalize the per-core Bass program."""
    import concourse.bacc as bacc
    import concourse.tile as tile
    from concourse import mybir
    from concourse.masks import make_identity

    f32 = mybir.dt.float32
    f32r = mybir.dt.float32r
    bf16 = mybir.dt.bfloat16
    AF = mybir.ActivationFunctionType
    AX = mybir.AxisListType
    MUL = mybir.AluOpType.mult

    nc = bacc.Bacc("TRN2", target_bir_lowering=False, debug=False)

    xT = nc.dram_tensor("xT", [KT, 128, BS], bf16, kind="ExternalInput")
    Wf = nc.dram_tensor("Wf", [KT, 128, NT], bf16, kind="ExternalInput")
    # leaf tables host-transposed to [d, (leaf, tree)] for contiguous DMA
    gt = nc.dram_tensor("gt", [LEAF_DIMS, N_LEAVES * NUM_TREES], bf16, kind="ExternalInput")
    lwt = nc.dram_tensor("lwt", [LEAF_DIMS, N_LEAVES * NUM_TREES], bf16, kind="ExternalInput")
    if use_bias:
        bias = nc.dram_tensor("bias", [1, NT], bf16, kind="ExternalInput")
    outT = nc.dram_tensor("outT", [LEAF_DIMS, BS], f32, kind="ExternalOutput")

    with tile.TileContext(nc) as tc:
        with tc.tile_pool(name="const", bufs=1) as cpool, \
             tc.tile_pool(name="wts", bufs=1) as wpool, \
             tc.tile_pool(name="psA", bufs=4, space="PSUM") as psA, \
             tc.tile_pool(name="psT", bufs=3, space="PSUM") as psT, \
             tc.tile_pool(name="psO", bufs=1, space="PSUM") as psO:

            ident = cpool.tile([128, 128], bf16, tag="ident")
            make_identity(nc, ident[:])

            # ---- input DMAs.  x + leaf tables on the ACT HW-DGE ring, the
            # (bigger) weight matrix on the SP ring, so they overlap. ----
            xk = []
            for k in range(KT):
                t = wpool.tile([128, BS], bf16, tag=f"xk{k}", name=f"xk{k}")
                nc.scalar.dma_start(t[:], xT[k, :, :])
                xk.append(t)

            wk = [wpool.tile([128, NT], bf16, tag=f"wk{k}", name=f"wk{k}") for k in range(KT)]
            for m in range(NCHUNK // 2):
                for k in range(KT):
                    nc.sync.dma_start(
                        wk[k][:, m * 2 * CHUNK:(m + 1) * 2 * CHUNK],
                        Wf[k, :, m * 2 * CHUNK:(m + 1) * 2 * CHUNK],
                    )
            if use_bias:
                bias_sb = cpool.tile([1, NT], bf16, tag="bias")
                nc.sync.dma_start(bias_sb[:], bias[:, :])
                ones1 = cpool.tile([1, 128], bf16, tag="ones1")
                nc.gpsimd.memset(ones1[:], 1.0)

            wsm = wpool.tile([128, N_LEAVES, NUM_TREES], bf16, tag="wsm")
            gtile = wpool.tile([128, N_LEAVES, NUM_TREES], bf16, tag="gtile")
            nc.sync.dma_start(gtile[:], gt[:, :].rearrange("d (l t) -> d l t", t=NUM_TREES))
            lwtile = wpool.tile([128, N_LEAVES, NUM_TREES], bf16, tag="lwtile")
            nc.sync.dma_start(lwtile[:], lwt[:, :].rearrange("d (l t) -> d l t", t=NUM_TREES))

            def emit_phase0_compute():
                # w = leaf_weight * softmax(gates): exp on ACT, reduce on DVE,
                # normalize on the idle Pool engine (PE transposes deferred)
                nc.scalar.activation(gtile[:], gtile[:], AF.Exp)
                ehalf = cpool.tile([128, N_LEAVES, NUM_TREES // 2], bf16, tag="ehalf")
                nc.vector.tensor_add(ehalf[:], gtile[:, :, 0:NUM_TREES // 2],
                                     gtile[:, :, NUM_TREES // 2:NUM_TREES])
                equar = cpool.tile([128, N_LEAVES, NUM_TREES // 4], bf16, tag="equar")
                nc.vector.tensor_add(equar[:], ehalf[:, :, 0:NUM_TREES // 4],
                                     ehalf[:, :, NUM_TREES // 4:NUM_TREES // 2])
                s = cpool.tile([128, N_LEAVES], f32, tag="s")
                nc.vector.reduce_sum(s[:], equar[:], axis=AX.X)
                r = cpool.tile([128, N_LEAVES], f32, tag="r")
                nc.vector.reciprocal(r[:], s[:])
                for l in range(N_LEAVES):
                    nc.vector.scalar_tensor_tensor(
                        wsm[:, l, :], gtile[:, l, :], r[:, l:l + 1], lwtile[:, l, :],
                        op0=MUL, op1=MUL,
                    )

            # ---- main loop: batch-tile pairs ----
            with tc.tile_pool(name="gp", bufs=1) as gpool, \
                 tc.tile_pool(name="pp", bufs=2) as ppool, \
                 tc.tile_pool(name="pfp", bufs=2) as pfpool, \
                 tc.tile_pool(name="ptp", bufs=1) as pTpool, \
                 tc.tile_pool(name="outp", bufs=1) as outpool:

                out_ps = psO.tile([LEAF_DIMS, BS], f32, tag="out_ps")
                out_sb = outpool.tile([LEAF_DIMS, BS], f32, tag="out_sb")

                # PE warm-up: dummy transposes into the (not yet used) output
                # PSUM bank while the first weight DMAs are in flight, so the
                # HAM clock gate is released before phase A starts
                ident32 = cpool.tile([128, 128], f32, tag="ident32")
                make_identity(nc, ident32[:])
                for _wi in range(10):
                    nc.tensor.transpose(out_ps[:, 0:128], ident32[:], ident32[:])

                def emit_phaseD_slice(i, width=1):
                    bsl = slice(i * 128, (i + width) * 128)
                    for j in range(NPT):
                        nc.tensor.matmul(out_ps[:, bsl],
                                         wT_all[:, j // 4, (j % 4) * 128:(j % 4 + 1) * 128],
                                         pT_all[:, j, bsl],
                                         start=(j == 0), stop=(j == NPT - 1))
                    nc.vector.tensor_copy(out_sb[:, bsl], out_ps[:, bsl])
                    nc.sync.dma_start(outT[:, bsl], out_sb[:, bsl])

                # pT_all[:, j, :] = chunk j of p_T, [(leaf,tree)(128), b(512)]
                pT_all = pTpool.tile([128, NPT, BS], bf16, tag="pT")
                # g_all[:, i, :] = sigmoid gates for batch tile i
                g_all = gpool.tile([128, NBT, NT], bf16, tag="g")
                wT_all = wpool.tile([128, NPT // 4, 512], bf16, tag="wT")

                for pair in range(NBT // 2):
                    i0 = 2 * pair
                    # phase A: one 2-bank PSUM tile serves both batch tiles of
                    # the pair; chunks consumed in DMA arrival order
                    for q in range(2):
                        bsl = slice((i0 + q) * 128, (i0 + q + 1) * 128)
                        for n in range(NCHUNK):
                            csl = slice(n * CHUNK, (n + 1) * CHUNK)
                            lg = psA.tile([128, CHUNK], f32, tag="lg")
                            for k in range(KT):
                                nc.tensor.matmul(
                                    lg[:], xk[k][:, bsl], wk[k][:, csl],
                                    start=(k == 0),
                                    stop=(k == KT - 1 and not use_bias),
                                )
                            if use_bias:
                                nc.tensor.matmul(
                                    lg[:], ones1[:], bias_sb[:, csl],
                                    start=False, stop=True,
                                )
                            nc.scalar.activation(g_all[:, i0 + q, csl], lg[:], AF.Sigmoid)

                    if pair == 1:
                        # phase 0 PE part here: wsm is ready by now and this
                        # keeps the post-loop PE tail short
                        for jj in range(NPT // 4):
                            tp4 = psT.tile([128, 512], bf16, tag="tp")
                            for qq in range(4):
                                j = 4 * jj + qq
                                nc.tensor.transpose(
                                    tp4[:, qq * 128:(qq + 1) * 128],
                                    wsm[:, 2 * j:2 * j + 2, :], ident[:])
                            nc.scalar.copy(wT_all[:, jj, :], tp4[:])

                        emit_phaseD_slice(0, width=2)

                    for q in range(2):
                        i = i0 + q
                        bsl = slice(i * 128, (i + 1) * 128)
                        # phase B: block-layout level doubling (DVE, 2x mode)
                        pa = ppool.tile([128, 2048], bf16, tag="pa")
                        pb = ppool.tile([128, 2048], bf16, tag="pb")
                        pf = pfpool.tile([128, 4096], bf16, tag="pf")
                        # level 0: p = [g0 | 1-g0]
                        nc.vector.tensor_copy(pa[:, 0:64], g_all[:, i, 0:64])
                        nc.scalar.activation(pa[:, 64:128], g_all[:, i, 0:64],
                                             AF.Copy, bias=1.0, scale=-1.0)
                        cur, other = pa, pb
                        for lvl in range(1, MAX_DEPTH):
                            h = (2 ** lvl) * 64
                            off = (2 ** lvl - 1) * 64
                            dst = pf if lvl == MAX_DEPTH - 1 else other
                            if lvl == MAX_DEPTH - 1:
                                # half-split so the first transpose chunks can
                                # start while the second half still computes
                                hh = h // 2
                                nc.vector.tensor_mul(dst[:, 0:hh], cur[:, 0:hh],
                                                     g_all[:, i, off:off + hh])
                                nc.vector.tensor_sub(dst[:, h:h + hh], cur[:, 0:hh],
                                                     dst[:, 0:hh])
                                nc.vector.tensor_mul(dst[:, hh:h], cur[:, hh:h],
                                                     g_all[:, i, off + hh:off + h])
                                nc.vector.tensor_sub(dst[:, h + hh:2 * h], cur[:, hh:h],
                                                     dst[:, hh:h])
                            else:
                                nc.vector.tensor_mul(dst[:, 0:h], cur[:, 0:h],
                                                     g_all[:, i, off:off + h])
                                nc.vector.tensor_sub(dst[:, h:2 * h], cur[:, 0:h],
                                                     dst[:, 0:h])
                            cur, other = dst, cur

                        # phase C: transpose p -> [(leaf,tree), b] bf16;
                        # 4 chunks share one PSUM bank, one DVE copy per bank
                        for jj in range(NPT // 4):
                            tp4 = psT.tile([128, 512], bf16, tag="tp")
                            for qq in range(4):
                                j = 4 * jj + qq
                                nc.tensor.transpose(
                                    tp4[:, qq * 128:(qq + 1) * 128],
                                    pf[:, j * 128:(j + 1) * 128], ident[:])
                            mod = 2 if pair == 1 else 3
                            if jj % mod == mod - 1:
                                nc.scalar.copy(
                                    pT_all[:, 4 * jj:4 * jj + 4, bsl], tp4[:])
                            else:
                                nc.vector.tensor_copy(
                                    pT_all[:, 4 * jj:4 * jj + 4, bsl], tp4[:])

                        if pair == 1:
                            emit_phaseD_slice(i)

                    if pair == 0:
                        emit_phase0_compute()



    nc.finalize()
    return nc


def _get_nc(use_bias: bool):
    if use_bias not in _BUILT:
        _BUILT[use_bias] = _build(use_bias)
    return _BUILT[use_bias]


def _make_in_maps(x, W, b, leaf_weight, gates):
    x = np.ascontiguousarray(np.asarray(x, dtype=np.float32))
    W = np.asarray(W, dtype=np.float32)
    b = np.asarray(b, dtype=np.float32)
    leaf_weight = np.asarray(leaf_weight, dtype=np.float32)
    gates = np.asarray(gates, dtype=np.float32)

    use_bias = bool(np.any(b))
    # host-side layout prep (permutations / transposes / bf16 cast)
    Wp = W[_NODES_PERM]                                   # [63, 512, 64]
    Wf = np.ascontiguousarray(
        Wp.transpose(1, 0, 2).reshape(KT, 128, NT).astype(BF16))
    # leaf tables -> [d, (leaf, tree)] so the DMA is contiguous per partition
    gt = np.ascontiguousarray(
        gates[_LEAF_PERM].transpose(1, 0, 2).reshape(LEAF_DIMS, -1).astype(BF16))
    lwt = np.ascontiguousarray(
        leaf_weight[_LEAF_PERM].transpose(1, 0, 2).reshape(LEAF_DIMS, -1).astype(BF16))
    if use_bias:
        bias = np.ascontiguousarray(b[_NODES_PERM].reshape(1, NT).astype(BF16))

    in_maps = []
    for c in range(N_CORES):
        xs = x[c * BS:(c + 1) * BS]                       # [512, 512]
        xTc = np.ascontiguousarray(xs.T.reshape(KT, 128, BS).astype(BF16))
        m = {"xT": xTc, "Wf": Wf, "gt": gt, "lwt": lwt}
        if use_bias:
            m["bias"] = bias
        in_maps.append(m)
    return use_bias, in_maps


def kernel(x, W, b, leaf_weight, gates):
    from concourse.bass_utils import run_bass_kernel_spmd

    use_bias, in_maps = _make_in_maps(x, W, b, leaf_weight, gates)
    nc = _get_nc(use_bias)

    res = run_bass_kernel_spmd(nc, in_maps, core_ids=list(range(N_CORES)))
    out = np.empty((BATCH, LEAF_DIMS), dtype=np.float32)
    for c in range(N_CORES):
        out[c * BS:(c + 1) * BS] = res.results[c]["outT"].T
    return out
